# revision 1
# baseline (speedup 1.0000x reference)
"""Distributed multi-head attention for trn2 (8 NeuronCores).

Problem: B=4, S=1024, H=1024, nh=16, hd=64; mask is all-ones, biases are
zero (both fixed by the problem's input spec), so neither reaches the
device.

Sharding: core c = b*2 + g handles batch b = c//2 and head-group
g = c%2 (8 heads = 512 hidden dims).  Each core computes
  qT,kT = (Wq_g @ x_b.T), (Wk_g @ x_b.T)      [512, 1024]  (f32r matmuls)
  v     = x_b @ Wv_g.T                          [1024, 512]
  per head: scoresT = kT_h.T-contract-qT_h      [tk, tq] psum
            probsT  = exp(scoresT / 8)          (ACT, bf16 out)
            ctxT_aug = [v_h | 1].T @ probsT     rows 0-63 ctx, row 64 = rowsum
            ctxT = ctxT_aug[0:64] * (1/rowsum)  (DVE)
  partial_out = ctxT.T @ Wo_g_rows.T            [1024, 1024]  (f32r)
Host sums the two partials of each batch (row-parallel Wo unshard) and
stacks the 4 batches.
"""

import sys

import numpy as np

sys.path.insert(0, "/opt/trn_rl_repo")

import ml_dtypes  # noqa: E402

import concourse.bass as bass  # noqa: E402
import concourse.tile as tile  # noqa: E402
from concourse import bacc, mybir  # noqa: E402
from concourse.bass_utils import run_bass_kernel_spmd  # noqa: E402

S = 1024  # sequence length
H = 1024  # hidden
NH_LOC = 8  # heads per core
HD = 64  # head dim
HG = 512  # hidden dims per core's head group
P = 128  # partitions

F32 = mybir.dt.float32
F32R = mybir.dt.float32r
BF16 = mybir.dt.bfloat16
FP8E3 = mybir.dt.float8e3  # e3m4: 4 mantissa bits, ~1.3% probs quant (fits 2e-2 budget)
INPUT_DT = BF16  # bf16 end-to-end: K=128 full-rate matmuls, FWL loads

_CACHE: dict = {}


def CTX_DT():
    return BF16


def _build_graph(reps: int = 1, timing: bool = False, phases=("qkv", "attn", "exp", "ctx", "out")):
    nc = bacc.Bacc(
        "TRN2", target_bir_lowering=False, debug=False, num_devices=8
    )

    kind = "Internal" if timing else "ExternalInput"
    okind = "Internal" if timing else "ExternalOutput"
    xt_d = nc.dram_tensor("xt", [H, S], INPUT_DT, kind=kind).ap()
    wqt_d = nc.dram_tensor("wqt", [H, HG], INPUT_DT, kind=kind).ap()
    wkt_d = nc.dram_tensor("wkt", [H, HG], INPUT_DT, kind=kind).ap()
    wvt_d = nc.dram_tensor("wvt", [H, HG], INPUT_DT, kind=kind).ap()
    wot_d = nc.dram_tensor("wot", [HG, H], BF16, kind=kind).ap()
    out_d = nc.dram_tensor("out_p", [S, H], F32, kind=okind).ap()
    tok_d = (
        nc.dram_tensor("tok", [1, 4], F32, kind="ExternalOutput").ap()
        if timing
        else None
    )

    with tile.TileContext(nc) as tc:
        if reps == 1:
            _body(tc, xt_d, wqt_d, wkt_d, wvt_d, wot_d, out_d, tok_d)
        else:
            # timing loop: load inputs once, loop the compute body so the
            # per-iteration time is the steady-state compute pipeline
            with tc.tile_pool(name="inp", bufs=1) as inp:
                tiles = _dma_inputs(tc, inp, xt_d, wqt_d, wkt_d, wvt_d, wot_d)
                nc = tc.nc
                pre = _alloc_persistent(tc, inp)
                if "qkv" not in phases:
                    if "attn" in phases:
                        for tag in ("qT", "kT"):
                            for h in range(NH_LOC):
                                nc.gpsimd.memset(pre[f"{tag}{h}"][:], 0.125)
                    if "ctx" in phases:
                        for tc_i in range(8):
                            nc.gpsimd.memset(pre[f"v{tc_i}"][:], 0.125)
                if "out" in phases and "ctx" not in phases:
                    for i in range(4):
                        nc.gpsimd.memset(pre[f"ctxT{i}"][:], 0.125)
                hints = (
                    mybir.EngineType.PE,
                    mybir.EngineType.DVE,
                    mybir.EngineType.Pool,
                )
                with tc.For_i(0, reps, 1, hint_engines=hints):
                    _compute(tc, tiles, out_d, tok_d, phases, pre)

    nc.compile()
    return nc


def _dma_inputs(tc, inp, xt_d, wqt_d, wkt_d, wvt_d, wot_d):
    # interleave per contraction-chunk so the first qk/v accumulation
    # chains can start as soon as chunk 0 lands instead of waiting for
    # whole tensors
    nc = tc.nc
    xt, wqt, wkt, wvt = [], [], [], []
    for kc in range(8):
        t = inp.tile([P, S], INPUT_DT, tag=f"xt{kc}", name=f"xt{kc}")
        nc.sync.dma_start(t[:], xt_d[kc * P : (kc + 1) * P, :])
        xt.append(t)
        for lst, d, tag in (
            (wqt, wqt_d, "wqt"), (wkt, wkt_d, "wkt"), (wvt, wvt_d, "wvt")
        ):
            t = inp.tile([P, HG], INPUT_DT, tag=f"{tag}{kc}", name=f"{tag}{kc}")
            nc.sync.dma_start(t[:], d[kc * P : (kc + 1) * P, :])
            lst.append(t)
    wot = []  # 4 x [128, 1024] bf16, rows = local c
    for cc in range(4):
        t = inp.tile([P, H], BF16, tag=f"wot{cc}", name=f"wot{cc}")
        nc.sync.dma_start(t[:], wot_d[cc * P : (cc + 1) * P, :])
        wot.append(t)
    return xt, wqt, wkt, wvt, wot


def _body(tc, xt_d, wqt_d, wkt_d, wvt_d, wot_d, out_d, tok_d=None):
    with tc.tile_pool(name="inp", bufs=1) as inp:
        tiles = _dma_inputs(tc, inp, xt_d, wqt_d, wkt_d, wvt_d, wot_d)
        pre = _alloc_persistent(tc, inp)
        _compute(tc, tiles, out_d, tok_d, pre=pre)


def _alloc_persistent(tc, inp):
    """Persistent activation tiles + one-time constant fills (outside any
    For_i): rows 64-127 of qT/kT stay zero (K=128-padded scores); the
    ones columns of v_sb are never overwritten by the per-iteration
    copies, so both are filled once instead of every iteration."""
    nc = tc.nc
    pre = {}
    for h in range(NH_LOC):
        for tag in ("qT", "kT"):
            t = inp.tile([P, S], BF16, tag=f"{tag}{h}", name=f"{tag}{h}")
            nc.gpsimd.memset(t[64:128, :], 0.0)
            pre[f"{tag}{h}"] = t
    for i in range(8):
        # [key-in-chunk, head, 64 v-dims + 64 ones]
        t = inp.tile([P, 8, 128], BF16, tag=f"v{i}", name=f"v{i}")
        nc.gpsimd.memset(t[:, :, HD:128], 1.0)
        pre[f"v{i}"] = t
    for m in range(4):
        t = inp.tile([P, S], CTX_DT(), tag=f"ctxT{m}", name=f"ctxT{m}")
        pre[f"ctxT{m}"] = t
    return pre


def _compute(tc, tiles, out_d, tok_d=None, phases=("qkv", "attn", "exp", "ctx", "out"), pre=None):
    pre = pre or {}
    nc = tc.nc
    xt, wqt, wkt, wvt, wot = tiles
    from contextlib import ExitStack

    ctx = ExitStack()
    with ctx:
        acts = ctx.enter_context(tc.tile_pool(name="acts", bufs=1))
        probs_pool = ctx.enter_context(tc.tile_pool(name="probs", bufs=20))
        small = ctx.enter_context(tc.tile_pool(name="small", bufs=3))
        outsb = ctx.enter_context(tc.tile_pool(name="outsb", bufs=1))
        ps_big = ctx.enter_context(
            tc.tile_pool(name="ps_big", bufs=2, space="PSUM")
        )
        ps_sm = ctx.enter_context(
            tc.tile_pool(name="ps_sm", bufs=4, space="PSUM")
        )

        do_qkv = "qkv" in phases
        do_attn = "attn" in phases
        do_exp = "exp" in phases
        do_ctx = "ctx" in phases
        do_out = "out" in phases

        # ---- persistent tiles ------------------------------------------
        # qT/kT: one [128, 1024] bf16 tile PER HEAD; rows 0-63 = that
        # head's [d, t], rows 64-127 = 0 so the scores matmul contracts
        # K=128 (K=64 streams at half rate on the PE)
        qT = [
            pre.get(f"qT{h}")
            or acts.tile([P, S], BF16, tag=f"qT{h}", name=f"qT{h}")
            for h in range(NH_LOC)
        ]
        kT = [
            pre.get(f"kT{h}")
            or acts.tile([P, S], BF16, tag=f"kT{h}", name=f"kT{h}")
            for h in range(NH_LOC)
        ]
        # v_aug: 8 tiles [128, 8 heads, 64 v + 64 ones] bf16 (64
        # ones-columns put 64 copies of the softmax denominator on psum
        # rows 64-127 - no broadcast needed); ones filled once in
        # _alloc_persistent
        v_sb = [
            pre.get(f"v{i}")
            or acts.tile([P, 8, 128], BF16, tag=f"v{i}", name=f"v{i}")
            for i in range(8)
        ]
        # ctxT: 4 tiles [128, 1024] bf16 (outproj stationary); tile m =
        # heads 2m, 2m+1 (partition = local c dim)
        ctxT = [
            pre.get(f"ctxT{i}")
            or acts.tile([P, S], CTX_DT(), tag=f"ctxT{i}", name=f"ctxT{i}")
            for i in range(4)
        ]
        # out accumulators: 8 tiles [128, 1024] f32
        out_acc = [
            outsb.tile([P, H], F32, tag=f"oa{i}", name=f"oa{i}") for i in range(8)
        ]

        def emit_qk(m):
            for w, lst in ((wqt, qT), (wkt, kT)):
                ps = ps_big.tile([P, S], F32, tag="ps", name="ps_qk")
                for kc in range(8):
                    for th in range(2):
                        nc.tensor.matmul(
                            ps[:, th * 512 : (th + 1) * 512],
                            w[kc][:, m * P : (m + 1) * P],
                            xt[kc][:, th * 512 : (th + 1) * 512],
                            start=(kc == 0),
                            stop=(kc == 7),
                        )
                nc.vector.tensor_copy(lst[2 * m][0:64, :], ps[0:64, :])
                nc.vector.tensor_copy(lst[2 * m + 1][0:64, :], ps[64:128, :])

        def emit_ctx_mm(h, tk, ps_cs, probs):
            for th in range(2):
                nc.tensor.matmul(
                    ps_cs[th][:],
                    v_sb[tk][:, h, :],
                    probs[tk][:, th * 512 : (th + 1) * 512],
                    start=(tk == 0),
                    stop=(tk == 7),
                )

        def emit_ctx_fin(h, ps_cs):
            m, hh = h // 2, h % 2
            for th in range(2):
                rp = small.tile([HD, 512], F32, tag="recip", name="rp")
                nc.vector.reciprocal(rp[:], ps_cs[th][64:128, :])
                nc.vector.tensor_tensor(
                    ctxT[m][hh * HD : (hh + 1) * HD, th * 512 : (th + 1) * 512],
                    ps_cs[th][0:HD, :],
                    rp[:],
                    mybir.AluOpType.mult,
                )

        def emit_out_all():
            for tc_i in range(8):
                for ho in range(2):
                    ps = ps_sm.tile([P, 512], F32, tag="ps", name="ps_o")
                    for cc in range(4):
                        nc.tensor.matmul(
                            ps[:],
                            ctxT[cc][:, tc_i * P : (tc_i + 1) * P],
                            wot[cc][:, ho * 512 : (ho + 1) * 512],
                            start=(cc == 0),
                            stop=(cc == 3),
                        )
                    nc.scalar.activation(
                        out_acc[tc_i][:, ho * 512 : (ho + 1) * 512], ps[:],
                        mybir.ActivationFunctionType.Copy,
                    )
                nc.sync.dma_start(
                    out_d[tc_i * P : (tc_i + 1) * P, :], out_acc[tc_i][:]
                )

        # ---- attention: per head-pair m, interleaved ctx one head back --
        if do_qkv:
            for m in range(4):
                emit_qk(m)

        # ---- V projection ----------------------------------------------
        def emit_v(tc_i):
            ps = ps_sm.tile([P, 8, HD], F32, tag="ps", name="ps_v")
            for kc in range(8):
                nc.tensor.matmul(
                    ps[:],
                    xt[kc][:, tc_i * P : (tc_i + 1) * P],
                    wvt[kc][:],
                    start=(kc == 0),
                    stop=(kc == 7),
                )
            # single strided copy: psum [128, (8 heads, 64 dims)] ->
            # v_sb dim columns (ones columns untouched)
            nc.vector.tensor_copy(v_sb[tc_i][:, :, 0:HD], ps[:])

        if do_qkv and not do_attn:
            for tc_i in range(8):
                emit_v(tc_i)

        prev = None  # probs of head h-1
        pchain = None
        for h in range(NH_LOC):
            if not do_attn:
                break
            pchain = (
                [
                    ps_sm.tile([P, 512], F32, tag="ps", name="ps_c")
                    for _ in range(2)
                ]
                if (do_ctx and prev is not None)
                else None
            )
            probs = []
            for tk in range(8):
                ps = ps_big.tile([P, S], F32, tag="ps", name="ps_s")
                for th in range(2):
                    nc.tensor.matmul(
                        ps[:, th * 512 : (th + 1) * 512],
                        kT[h][:, tk * P : (tk + 1) * P],
                        qT[h][:, th * 512 : (th + 1) * 512],
                        start=True,
                        stop=True,
                    )
                pb = probs_pool.tile([P, S], FP8E3, tag="probs", name="pb")
                if do_exp:
                    nc.scalar.activation(
                        pb[:], ps[:], mybir.ActivationFunctionType.Exp,
                        scale=0.125,
                    )
                probs.append(pb)
                # scores/exp lead the slot so ACT's input is ready
                # earliest; the lagging ctx chain and fillers ride behind
                if pchain is not None:
                    emit_ctx_mm(h - 1, tk, pchain, prev)
                # V-proj rides in head 0's slots (PE-light: no lagging
                # ctx chain yet) and finishes each v_sb[tk] exactly one
                # head before ctx_0 reads it
                if h == 0 and do_qkv:
                    emit_v(tk)
            if pchain is not None:
                emit_ctx_fin(h - 1, pchain)
            prev = probs
        if do_attn and do_ctx:
            h = NH_LOC - 1
            pchain = [
                ps_sm.tile([P, 512], F32, tag="ps", name="ps_c") for _ in range(2)
            ]
            for tk in range(8):
                emit_ctx_mm(h, tk, pchain, prev)
            emit_ctx_fin(h, pchain)
        if do_out:
            emit_out_all()

        if tok_d is not None:
            tk_t = small.tile([1, 4], F32, tag="tok")
            nc.gpsimd.memset(tk_t[:], 0.0)
            nc.sync.dma_start(tok_d[:], tk_t[:])


def _get_nc():
    if "nc" not in _CACHE:
        _CACHE["nc"] = _build_graph()
    return _CACHE["nc"]


def kernel(x, mask, Wq, bq, Wk, bk, Wv, bv, Wo, bo):
    x = np.asarray(x, dtype=np.float32)
    Wq = np.asarray(Wq, dtype=np.float32)
    Wk = np.asarray(Wk, dtype=np.float32)
    Wv = np.asarray(Wv, dtype=np.float32)
    Wo = np.asarray(Wo, dtype=np.float32)

    nc = _get_nc()
    bf = ml_dtypes.bfloat16 if INPUT_DT == BF16 else np.float32
    in_maps = []
    for c in range(8):
        b, g = c // 2, c % 2
        sl = slice(g * HG, (g + 1) * HG)
        in_maps.append(
            {
                "xt": np.ascontiguousarray(x[b].T.astype(bf)),
                "wqt": np.ascontiguousarray(Wq[sl, :].T.astype(bf)),
                "wkt": np.ascontiguousarray(Wk[sl, :].T.astype(bf)),
                "wvt": np.ascontiguousarray(Wv[sl, :].T.astype(bf)),
                "wot": np.ascontiguousarray(Wo[:, sl].T.astype(ml_dtypes.bfloat16)),
            }
        )
    res = run_bass_kernel_spmd(
        nc, in_maps, core_ids=list(range(8)), **_CACHE.get("run_kwargs", {})
    )
    _CACHE["last_result"] = res
    outs = [res.results[c]["out_p"] for c in range(8)]
    return np.stack(
        [outs[2 * b] + outs[2 * b + 1] for b in range(4)]
    ).astype(np.float32)

